# revision 17
# baseline (speedup 1.0000x reference)
"""Single-head attention (SEQ=8192, EMBED=2048, HEAD=128) on 8 TRN2 NeuronCores.

Sharding: queries (rows of Q / score matrix) are split 1024 rows per core.

Phase 1 (all bf16 on-chip): x and the projection weights are host-cast to
bf16; x^T is built with PE transposes (1 cycle/row in bf16) while the x row
blocks stream in. Each core projects its K shard, then V, then Q.

K/V exchange: three AllGathers (two K halves, then V) are fired as early
as the projections allow (~12us in), so the NRT rendezvous barrier and the
serialized ring transfers overlap the remaining projections and the Q pass.

Phase 2: scores are computed directly in transposed layout [t, sq] (t on
partitions) so the attention-weight matrix never needs an on-chip transpose
before the A@V matmul; softmax denominators come from DVE pair-sum trees plus
one ones-vector matmul per query group, and the 1/l scaling is applied to the
final [sq, h] tiles.

kernel(**inputs) takes the FULL unsharded inputs and returns the full output.
"""

import math

import numpy as np

import concourse.bacc as bacc
import concourse.mybir as mybir
import concourse.tile as tile
from concourse.bass_utils import run_bass_kernel_spmd
from concourse.masks import make_identity

SEQ, EMBED, HEAD = 8192, 2048, 128
NCORES = 8
P = 128

F32 = mybir.dt.float32
BF16 = mybir.dt.bfloat16

Id = mybir.ActivationFunctionType.Identity
Exp = mybir.ActivationFunctionType.Exp


def emit(nc, seq=SEQ, embed=EMBED, head=HEAD, ncores=NCORES):
    assert head == P
    s_loc = seq // ncores          # query rows per core
    e_ch = embed // P              # contraction chunks for the projections
    b_ch = s_loc // P              # 128-row blocks in the local shard
    n_half = s_loc // 2            # projection matmul free dim (512)
    assert n_half <= 512
    sq_g = min(256, s_loc)         # phase-2 query group (matmul free dim)
    n_g = s_loc // sq_g
    n_t = seq // P                 # key/value chunks
    quad = 4
    scale = 1.0 / math.sqrt(head)

    x = nc.dram_tensor("x", [s_loc, embed], BF16, kind="ExternalInput").ap()
    wq = nc.dram_tensor("wq", [embed, head], BF16, kind="ExternalInput").ap()
    wk = nc.dram_tensor("wk", [embed, head], BF16, kind="ExternalInput").ap()
    wv = nc.dram_tensor("wv", [embed, head], BF16, kind="ExternalInput").ap()
    bq = nc.dram_tensor("bq", [head], F32, kind="ExternalInput").ap()
    bk = nc.dram_tensor("bk", [head], F32, kind="ExternalInput").ap()
    bv = nc.dram_tensor("bv", [head], F32, kind="ExternalInput").ap()
    out = nc.dram_tensor("out", [s_loc, head], F32, kind="ExternalOutput").ap()

    with tile.TileContext(nc) as tc:
        with (
            tc.tile_pool(name="consts", bufs=1) as consts,
            tc.tile_pool(name="persist", bufs=1) as persist,
            tc.tile_pool(name="dram", bufs=1, space="DRAM") as dram,
        ):
            ident = consts.tile([P, P], F32)
            make_identity(nc, ident)
            ident_bf = consts.tile([P, P], BF16)
            nc.vector.tensor_copy(ident_bf[:], ident[:])
            ones_f32 = consts.tile([P, 1], F32)
            nc.vector.memset(ones_f32[:], 1.0)
            ones_col = consts.tile([P, 1], BF16)
            nc.vector.tensor_copy(ones_col[:], ones_f32[:])

            # persistent SBUF across the whole kernel
            qt_sb = persist.tile([P, s_loc], BF16)           # Q^T own shard
            kt_sb = persist.tile([P, n_t, P], BF16)          # K^T full
            v_sb = persist.tile([P, n_t, P], BF16)           # V natural full
            ksz = P * s_loc
            hsz = ksz // 2
            ag_k1_in = dram.tile([hsz], BF16)
            ag_k2_in = dram.tile([hsz], BF16)
            ag_v_in = dram.tile([ksz], BF16)
            ag_k1_out = dram.tile([ncores * hsz], BF16, addr_space="Shared")
            ag_k2_out = dram.tile([ncores * hsz], BF16, addr_space="Shared")
            ag_v_out = dram.tile([ncores * ksz], BF16, addr_space="Shared")

            # ---------------- Phase 1: project own shard ----------------
            with (
                tc.tile_pool(name="p1", bufs=1) as p1,
                tc.tile_pool(name="trps", bufs=3, space="PSUM") as trps,
                tc.tile_pool(name="projps", bufs=1, space="PSUM") as projps,
            ):
                wq_sb = p1.tile([P, e_ch, head], BF16)
                wk_sb = p1.tile([P, e_ch, head], BF16)
                wv_sb = p1.tile([P, e_ch, head], BF16)
                for w_sb, w_in in ((wk_sb, wk), (wv_sb, wv)):
                    nc.scalar.dma_start(
                        w_sb[:], w_in.rearrange("(c p) h -> p c h", p=P))
                bq_sb = p1.tile([P, 1], F32)
                bk_sb = p1.tile([P, 1], F32)
                bv_sb = p1.tile([P, 1], F32)
                nc.sync.dma_start(bq_sb[:], bq.unsqueeze(1))
                nc.sync.dma_start(bk_sb[:], bk.unsqueeze(1))
                nc.sync.dma_start(bv_sb[:], bv.unsqueeze(1))

                # x natural rows in, then x^T via PE transposes (bf16).
                x_b = x.rearrange("(b p) e -> b p e", p=P)
                x_sb = p1.tile([P, b_ch, embed], BF16)
                for b in range(b_ch):
                    eng = nc.sync if b % 2 == 0 else nc.scalar
                    eng.dma_start(x_sb[:, b, :], x_b[b])

                xt = p1.tile([P, e_ch, s_loc], BF16)
                for b in range(b_ch):
                    for eq in range(0, e_ch, 4):
                        tr = trps.tile([P, 4, P], BF16, tag="tr")
                        for j in range(4):
                            e = eq + j
                            nc.tensor.transpose(
                                tr[:, j, :],
                                x_sb[:, b, e * P:(e + 1) * P], ident_bf[:])
                        dst = xt[:, eq:eq + 4, b * P:(b + 1) * P]
                        if (b + eq // 4) % 2 == 0:
                            nc.vector.tensor_copy(dst, tr[:])
                        else:
                            nc.scalar.copy(dst, tr[:])

                kt_loc = p1.tile([P, s_loc], BF16)
                vt_loc = p1.tile([P, s_loc], BF16)

                def project_half(w_sb, b_sb, dst, h, tag):
                    hsl = slice(h * n_half, (h + 1) * n_half)
                    ps = projps.tile([P, n_half], F32, tag=tag)
                    for e in range(e_ch):
                        nc.tensor.matmul(
                            ps[:], w_sb[:, e, :], xt[:, e, hsl],
                            start=(e == 0), stop=(e == e_ch - 1))
                    nc.scalar.activation(dst[:, hsl], ps[:], Id,
                                         bias=b_sb[:, 0:1])

                # K first: each half feeds an early AllGather.
                for h, ag_in, ag_out in (
                    (0, ag_k1_in, ag_k1_out),
                    (1, ag_k2_in, ag_k2_out),
                ):
                    project_half(wk_sb, bk_sb, kt_loc, h, f"kps{h}")
                    hsl = slice(h * n_half, (h + 1) * n_half)
                    nc.sync.dma_start(
                        ag_in.rearrange("(p s) -> p s", p=P), kt_loc[:, hsl])
                    nc.gpsimd.collective_compute(
                        "AllGather", mybir.AluOpType.bypass,
                        replica_groups=[list(range(ncores))],
                        ins=[ag_in.opt()], outs=[ag_out.opt()])

                # V next; natural layout + its AllGather
                project_half(wv_sb, bv_sb, vt_loc, 0, "kps0")
                project_half(wv_sb, bv_sb, vt_loc, 1, "kps1")
                v_nat = p1.tile([P, b_ch, head], BF16)
                for b in range(b_ch):
                    tr2 = trps.tile([P, P], BF16, tag="tr")
                    nc.tensor.transpose(tr2[:], vt_loc[:, b * P:(b + 1) * P],
                                        ident_bf[:])
                    nc.vector.tensor_copy(v_nat[:, b, :], tr2[:])
                nc.sync.dma_start(
                    ag_v_in.rearrange("(b p h) -> p b h", p=P, h=head),
                    v_nat[:])
                nc.gpsimd.collective_compute(
                    "AllGather", mybir.AluOpType.bypass,
                    replica_groups=[list(range(ncores))],
                    ins=[ag_v_in.opt()], outs=[ag_v_out.opt()])

                # Q last (overlaps the exchanges)
                nc.scalar.dma_start(
                    wq_sb[:], wq.rearrange("(c p) h -> p c h", p=P))
                project_half(wq_sb, bq_sb, qt_sb, 0, "kps0")
                project_half(wq_sb, bq_sb, qt_sb, 1, "kps1")

            # unpack gathered K^T / V into SBUF. kt_sb is in GATHER order:
            # every rank's first half, then the second halves. cmap maps a
            # phase-2 slot to the original chunk id for v_sb.
            hb2 = b_ch // 2
            n_loc1 = ncores * hb2
            for r in range(ncores):
                nc.sync.dma_start(
                    kt_sb[:, r * hb2:(r + 1) * hb2, :],
                    ag_k1_out[r * hsz:(r + 1) * hsz].rearrange(
                        "(p b t) -> p b t", p=P, b=hb2, t=P))
            for r in range(ncores):
                nc.sync.dma_start(
                    kt_sb[:, n_loc1 + r * hb2:n_loc1 + (r + 1) * hb2, :],
                    ag_k2_out[r * hsz:(r + 1) * hsz].rearrange(
                        "(p b t) -> p b t", p=P, t=P))
            for r in range(ncores):
                nc.scalar.dma_start(
                    v_sb[:, r * b_ch:(r + 1) * b_ch, :],
                    ag_v_out[r * ksz:(r + 1) * ksz].rearrange(
                        "(b p h) -> p b h", p=P, h=head))
            cmap = ([r * b_ch + b for r in range(ncores)
                     for b in range(hb2)]
                    + [r * b_ch + b for r in range(ncores)
                       for b in range(hb2, b_ch)])

            # ---------------- Phase 2: attention ----------------
            # All groups defer their A@V matmuls; the first pair's AV quads
            # are then interleaved one-per-score-quad into the later groups'
            # score stream (scores are ACT-bound: each quad costs ~0.53us of
            # PE vs ~1.1us of exp, so the PE slack absorbs the AV work once
            # the V AllGather has landed). Only the last pair's AV runs as a
            # solid PE-only block at the end.
            with (
                tc.tile_pool(name="p2", bufs=1) as p2,
                tc.tile_pool(name="p2s", bufs=2) as p2s,
                tc.tile_pool(name="stps", bufs=2, space="PSUM") as stps,
                tc.tile_pool(name="avps", bufs=2, space="PSUM") as avps,
            ):
                n_pairs = n_t // 2
                n_quads = n_t // quad

                def make_group(g):
                    qg = qt_sb[:, g * sq_g:(g + 1) * sq_g]
                    pt = p2.tile([P, n_t, sq_g], BF16, tag="pt", bufs=4,
                                 name="pt")
                    ls = p2.tile([P, n_pairs, sq_g], BF16, tag="ls",
                                 bufs=2, name="ls")
                    l_ps = stps.tile([1, sq_g], F32, tag="lps", bufs=1,
                                     name="l_ps")
                    ot_ps = avps.tile([P, sq_g], F32, tag="ot", name="ot_ps")

                    def sc(q, pt=pt, qg=qg):
                        # one score quad: 4 matmuls + one wide exp
                        cc = q * quad
                        st_ps = stps.tile([P, quad, sq_g], F32, tag="st",
                                          name="st_ps")
                        for k in range(quad):
                            nc.tensor.matmul(
                                st_ps[:, k, :], kt_sb[:, cc + k, :], qg,
                                start=True, stop=True,
                                skip_group_check=True)
                        nc.scalar.activation(pt[:, cc:cc + quad, :],
                                             st_ps[:], Exp, scale=scale)

                    def av(q, pt=pt, ls=ls, ot_ps=ot_ps):
                        # one AV quad (+ its DVE pair-sum every other quad)
                        cc = q * quad
                        if (cc // quad) % 2 == 1:
                            nc.vector.tensor_tensor(
                                ls[:, (cc - quad) // 2:
                                   (cc - quad) // 2 + quad, :],
                                pt[:, cc - quad:cc, :], pt[:, cc:cc + quad, :],
                                mybir.AluOpType.add)
                        for k in range(quad):
                            c = cc + k
                            nc.tensor.matmul(
                                ot_ps[:], v_sb[:, cmap[c], :], pt[:, c, :],
                                start=(c == 0), stop=(c == n_t - 1),
                                skip_group_check=True)

                    def fin(g=g, ls=ls, l_ps=l_ps, ot_ps=ot_ps):
                        w = n_pairs
                        while w > 1:
                            nc.vector.tensor_tensor(
                                ls[:, 0:w // 2, :], ls[:, 0:w // 2, :],
                                ls[:, w // 2:w, :],
                                mybir.AluOpType.add)
                            w //= 2
                        nc.tensor.matmul(
                            l_ps[:], ones_col[:], ls[:, 0, :],
                            start=True, stop=True, skip_group_check=True)
                        # 1/l as a per-partition column, then scale +
                        # transpose out
                        l_sb = p2s.tile([1, sq_g], F32, tag="lsb")
                        nc.vector.tensor_copy(l_sb[:], l_ps[:])
                        ot_sb = p2s.tile([P, sq_g], F32, tag="otsb")
                        nc.vector.tensor_copy(ot_sb[:], ot_ps[:])
                        for j in range(sq_g // P):
                            lc_ps = stps.tile([P, 1], F32, tag="st",
                                              name="lc_ps")
                            nc.tensor.transpose(
                                lc_ps[:], l_sb[0:1, j * P:(j + 1) * P],
                                ident[0:1, 0:1])
                            r_col = p2s.tile([P, 1], F32, tag="rcol",
                                             name="r_col")
                            nc.vector.reciprocal(r_col[:], lc_ps[:])
                            o_tr = stps.tile([P, P], F32, tag="st",
                                             name="o_tr")
                            nc.tensor.transpose(
                                o_tr[:], ot_sb[:, j * P:(j + 1) * P],
                                ident[:])
                            o_sb = p2s.tile([P, head], F32, tag="osb",
                                            name="o_sb")
                            nc.vector.tensor_scalar_mul(
                                o_sb[:], o_tr[:], r_col[:, 0:1])
                            row0 = g * sq_g + j * P
                            nc.sync.dma_start(out[row0:row0 + P, :],
                                              o_sb[:])

                    return sc, av, fin

                assert n_g == 4 and n_quads == 16
                hq = n_quads // 2
                grp = [make_group(g) for g in range(n_g)]
                (sa, aa, fa), (sb, ab, fb) = grp[0], grp[1]
                (scg, ac, fc), (sd, ad, fd) = grp[2], grp[3]
                # pair-1 scores (interleaved halves pipeline PE vs ACT)
                for q in range(hq):
                    sa(q)
                for q in range(hq):
                    sb(q)
                for q in range(hq, n_quads):
                    sa(q)
                for q in range(hq, n_quads):
                    sb(q)
                # g2 first half: V lands somewhere in here
                for q in range(hq):
                    scg(q)
                # remaining scores with pair-1 AV interleaved 1:1
                avq = [(aa, q) for q in range(n_quads)] + \
                    [(ab, q) for q in range(n_quads)]
                for s_fn, q0 in ((sd, 0), (scg, hq), (sd, hq)):
                    for q in range(q0, q0 + hq):
                        s_fn(q)
                        if avq:
                            f, i = avq.pop(0)
                            f(i)
                for f, i in avq:
                    f(i)
                fa()
                fb()
                # pair-2 AV + finishes
                for q in range(n_quads):
                    ac(q)
                for q in range(n_quads):
                    ad(q)
                fc()
                fd()
    nc.compile()
    return nc


_CACHE = {}


def _get_nc():
    if "nc" not in _CACHE:
        nc = bacc.Bacc("TRN2", target_bir_lowering=False, debug=False,
                       num_devices=NCORES)
        _CACHE["nc"] = emit(nc)
    return _CACHE["nc"]


def make_in_maps(x, Wq, bq, Wk, bk, Wv, bv):
    import ml_dtypes
    bf = ml_dtypes.bfloat16
    x = np.ascontiguousarray(np.asarray(x, dtype=np.float32).astype(bf))
    Wq = np.ascontiguousarray(np.asarray(Wq, dtype=np.float32).astype(bf))
    Wk = np.ascontiguousarray(np.asarray(Wk, dtype=np.float32).astype(bf))
    Wv = np.ascontiguousarray(np.asarray(Wv, dtype=np.float32).astype(bf))
    bq = np.ascontiguousarray(np.asarray(bq, dtype=np.float32))
    bk = np.ascontiguousarray(np.asarray(bk, dtype=np.float32))
    bv = np.ascontiguousarray(np.asarray(bv, dtype=np.float32))
    s_loc = SEQ // NCORES
    return [
        {
            "x": np.ascontiguousarray(x[c * s_loc:(c + 1) * s_loc]),
            "wq": Wq, "wk": Wk, "wv": Wv,
            "bq": bq, "bk": bk, "bv": bv,
        }
        for c in range(NCORES)
    ]


def kernel(x, Wq, bq, Wk, bk, Wv, bv):
    in_maps = make_in_maps(x, Wq, bq, Wk, bk, Wv, bv)
    res = run_bass_kernel_spmd(_get_nc(), in_maps,
                               core_ids=list(range(NCORES)))
    return np.concatenate(
        [res.results[c]["out"] for c in range(NCORES)], axis=0)


# revision 18
# speedup vs baseline: 1.0862x; 1.0862x over previous
"""Single-head attention (SEQ=8192, EMBED=2048, HEAD=128) on 8 TRN2 NeuronCores.

Sharding: queries (rows of Q / score matrix) are split 1024 rows per core.

Phase 1 (all bf16 on-chip): x and the projection weights are host-cast to
bf16; x^T is built with PE transposes (1 cycle/row in bf16) while the x row
blocks stream in. Each core projects its K shard, then V, then Q.

K/V exchange: three AllGathers (two K halves, then V) are fired as early
as the projections allow (~12us in), so the NRT rendezvous barrier and the
serialized ring transfers overlap the remaining projections and the Q pass.

Phase 2: scores are computed directly in transposed layout [t, sq] (t on
partitions) so the attention-weight matrix never needs an on-chip transpose
before the A@V matmul; softmax denominators come from DVE pair-sum trees plus
one ones-vector matmul per query group, and the 1/l scaling is applied to the
final [sq, h] tiles.

kernel(**inputs) takes the FULL unsharded inputs and returns the full output.
"""

import math

import numpy as np

import concourse.bacc as bacc
import concourse.mybir as mybir
import concourse.tile as tile
from concourse.bass_utils import run_bass_kernel_spmd
from concourse.masks import make_identity

SEQ, EMBED, HEAD = 8192, 2048, 128
NCORES = 8
P = 128

F32 = mybir.dt.float32
BF16 = mybir.dt.bfloat16

Id = mybir.ActivationFunctionType.Identity
Exp = mybir.ActivationFunctionType.Exp


def emit(nc, seq=SEQ, embed=EMBED, head=HEAD, ncores=NCORES):
    assert head == P
    s_loc = seq // ncores          # query rows per core
    e_ch = embed // P              # contraction chunks for the projections
    b_ch = s_loc // P              # 128-row blocks in the local shard
    n_half = s_loc // 2            # projection matmul free dim (512)
    assert n_half <= 512
    sq_g = min(256, s_loc)         # phase-2 query group (matmul free dim)
    n_g = s_loc // sq_g
    n_t = seq // P                 # key/value chunks
    quad = 4
    scale = 1.0 / math.sqrt(head)

    x = nc.dram_tensor("x", [s_loc, embed], BF16, kind="ExternalInput").ap()
    wq = nc.dram_tensor("wq", [embed, head], BF16, kind="ExternalInput").ap()
    wk = nc.dram_tensor("wk", [embed, head], BF16, kind="ExternalInput").ap()
    wv = nc.dram_tensor("wv", [embed, head], BF16, kind="ExternalInput").ap()
    bq = nc.dram_tensor("bq", [head], F32, kind="ExternalInput").ap()
    bk = nc.dram_tensor("bk", [head], F32, kind="ExternalInput").ap()
    bv = nc.dram_tensor("bv", [head], F32, kind="ExternalInput").ap()
    out = nc.dram_tensor("out", [s_loc, head], F32, kind="ExternalOutput").ap()

    with tile.TileContext(nc) as tc:
        with (
            tc.tile_pool(name="consts", bufs=1) as consts,
            tc.tile_pool(name="persist", bufs=1) as persist,
            tc.tile_pool(name="dram", bufs=1, space="DRAM") as dram,
        ):
            ident = consts.tile([P, P], F32)
            make_identity(nc, ident)
            ident_bf = consts.tile([P, P], BF16)
            nc.vector.tensor_copy(ident_bf[:], ident[:])
            ones_f32 = consts.tile([P, 1], F32)
            nc.vector.memset(ones_f32[:], 1.0)
            ones_col = consts.tile([P, 1], BF16)
            nc.vector.tensor_copy(ones_col[:], ones_f32[:])

            # persistent SBUF across the whole kernel
            qt_sb = persist.tile([P, s_loc], BF16)           # Q^T own shard
            kt_sb = persist.tile([P, n_t, P], BF16)          # K^T full
            v_sb = persist.tile([P, n_t, P], BF16)           # V natural full
            ksz = P * s_loc
            hsz = ksz // 2
            ag_k1_in = dram.tile([hsz], BF16)
            ag_k2_in = dram.tile([hsz], BF16)
            ag_v_in = dram.tile([ksz], BF16)
            ag_k1_out = dram.tile([ncores * hsz], BF16, addr_space="Shared")
            ag_k2_out = dram.tile([ncores * hsz], BF16, addr_space="Shared")
            ag_v_out = dram.tile([ncores * ksz], BF16, addr_space="Shared")

            # ---------------- Phase 1: project own shard ----------------
            with (
                tc.tile_pool(name="p1", bufs=1) as p1,
                tc.tile_pool(name="trps", bufs=3, space="PSUM") as trps,
                tc.tile_pool(name="projps", bufs=1, space="PSUM") as projps,
            ):
                wq_sb = p1.tile([P, e_ch, head], BF16)
                wk_sb = p1.tile([P, e_ch, head], BF16)
                wv_sb = p1.tile([P, e_ch, head], BF16)
                for w_sb, w_in in ((wk_sb, wk), (wv_sb, wv)):
                    nc.scalar.dma_start(
                        w_sb[:], w_in.rearrange("(c p) h -> p c h", p=P))
                bq_sb = p1.tile([P, 1], F32)
                bk_sb = p1.tile([P, 1], F32)
                bv_sb = p1.tile([P, 1], F32)
                nc.sync.dma_start(bq_sb[:], bq.unsqueeze(1))
                nc.sync.dma_start(bk_sb[:], bk.unsqueeze(1))
                nc.sync.dma_start(bv_sb[:], bv.unsqueeze(1))

                # x natural rows in, then x^T via PE transposes (bf16).
                x_b = x.rearrange("(b p) e -> b p e", p=P)
                x_sb = p1.tile([P, b_ch, embed], BF16)
                for b in range(b_ch):
                    eng = nc.sync if b % 2 == 0 else nc.scalar
                    eng.dma_start(x_sb[:, b, :], x_b[b])

                xt = p1.tile([P, e_ch, s_loc], BF16)
                for b in range(b_ch):
                    for eq in range(0, e_ch, 4):
                        tr = trps.tile([P, 4, P], BF16, tag="tr")
                        for j in range(4):
                            e = eq + j
                            nc.tensor.transpose(
                                tr[:, j, :],
                                x_sb[:, b, e * P:(e + 1) * P], ident_bf[:])
                        dst = xt[:, eq:eq + 4, b * P:(b + 1) * P]
                        if (b + eq // 4) % 2 == 0:
                            nc.vector.tensor_copy(dst, tr[:])
                        else:
                            nc.scalar.copy(dst, tr[:])

                kt_loc = p1.tile([P, s_loc], BF16)
                vt_loc = p1.tile([P, s_loc], BF16)

                def project_half(w_sb, b_sb, dst, h, tag):
                    hsl = slice(h * n_half, (h + 1) * n_half)
                    ps = projps.tile([P, n_half], F32, tag=tag)
                    for e in range(e_ch):
                        nc.tensor.matmul(
                            ps[:], w_sb[:, e, :], xt[:, e, hsl],
                            start=(e == 0), stop=(e == e_ch - 1))
                    nc.scalar.activation(dst[:, hsl], ps[:], Id,
                                         bias=b_sb[:, 0:1])

                # K first: each half feeds an early AllGather.
                for h, ag_in, ag_out in (
                    (0, ag_k1_in, ag_k1_out),
                    (1, ag_k2_in, ag_k2_out),
                ):
                    project_half(wk_sb, bk_sb, kt_loc, h, f"kps{h}")
                    hsl = slice(h * n_half, (h + 1) * n_half)
                    nc.sync.dma_start(
                        ag_in.rearrange("(p s) -> p s", p=P), kt_loc[:, hsl])
                    nc.gpsimd.collective_compute(
                        "AllGather", mybir.AluOpType.bypass,
                        replica_groups=[list(range(ncores))],
                        ins=[ag_in.opt()], outs=[ag_out.opt()])

                # V next; natural layout + its AllGather
                project_half(wv_sb, bv_sb, vt_loc, 0, "kps0")
                project_half(wv_sb, bv_sb, vt_loc, 1, "kps1")
                v_nat = p1.tile([P, b_ch, head], BF16)
                for b in range(b_ch):
                    tr2 = trps.tile([P, P], BF16, tag="tr")
                    nc.tensor.transpose(tr2[:], vt_loc[:, b * P:(b + 1) * P],
                                        ident_bf[:])
                    nc.vector.tensor_copy(v_nat[:, b, :], tr2[:])
                nc.sync.dma_start(
                    ag_v_in.rearrange("(b p h) -> p b h", p=P, h=head),
                    v_nat[:])
                nc.gpsimd.collective_compute(
                    "AllGather", mybir.AluOpType.bypass,
                    replica_groups=[list(range(ncores))],
                    ins=[ag_v_in.opt()], outs=[ag_v_out.opt()])

                # Q last (overlaps the exchanges)
                nc.scalar.dma_start(
                    wq_sb[:], wq.rearrange("(c p) h -> p c h", p=P))
                project_half(wq_sb, bq_sb, qt_sb, 0, "kps0")
                project_half(wq_sb, bq_sb, qt_sb, 1, "kps1")

            # unpack gathered K^T / V into SBUF. kt_sb is in GATHER order:
            # every rank's first half, then the second halves. cmap maps a
            # phase-2 slot to the original chunk id for v_sb.
            hb2 = b_ch // 2
            n_loc1 = ncores * hb2
            for r in range(ncores):
                nc.sync.dma_start(
                    kt_sb[:, r * hb2:(r + 1) * hb2, :],
                    ag_k1_out[r * hsz:(r + 1) * hsz].rearrange(
                        "(p b t) -> p b t", p=P, b=hb2, t=P))
            for r in range(ncores):
                nc.sync.dma_start(
                    kt_sb[:, n_loc1 + r * hb2:n_loc1 + (r + 1) * hb2, :],
                    ag_k2_out[r * hsz:(r + 1) * hsz].rearrange(
                        "(p b t) -> p b t", p=P, t=P))
            for r in range(ncores):
                nc.sync.dma_start(
                    v_sb[:, r * b_ch:(r + 1) * b_ch, :],
                    ag_v_out[r * ksz:(r + 1) * ksz].rearrange(
                        "(b p h) -> p b h", p=P, h=head))
            cmap = ([r * b_ch + b for r in range(ncores)
                     for b in range(hb2)]
                    + [r * b_ch + b for r in range(ncores)
                       for b in range(hb2, b_ch)])

            # ---------------- Phase 2: attention ----------------
            # All groups defer their A@V matmuls; the first pair's AV quads
            # are then interleaved one-per-score-quad into the later groups'
            # score stream (scores are ACT-bound: each quad costs ~0.53us of
            # PE vs ~1.1us of exp, so the PE slack absorbs the AV work once
            # the V AllGather has landed). Only the last pair's AV runs as a
            # solid PE-only block at the end.
            with (
                tc.tile_pool(name="p2", bufs=1) as p2,
                tc.tile_pool(name="p2s", bufs=2) as p2s,
                tc.tile_pool(name="stps", bufs=2, space="PSUM") as stps,
                tc.tile_pool(name="avps", bufs=2, space="PSUM") as avps,
            ):
                n_pairs = n_t // 2
                n_quads = n_t // quad

                def make_group(g):
                    qg = qt_sb[:, g * sq_g:(g + 1) * sq_g]
                    pt = p2.tile([P, n_t, sq_g], BF16, tag="pt", bufs=4,
                                 name="pt")
                    ls = p2.tile([P, n_pairs, sq_g], BF16, tag="ls",
                                 bufs=2, name="ls")
                    l_ps = stps.tile([1, sq_g], F32, tag="lps", bufs=1,
                                     name="l_ps")
                    ot_ps = avps.tile([P, sq_g], F32, tag="ot", name="ot_ps")

                    def sc(q, pt=pt, qg=qg):
                        # one score quad: 4 matmuls + one wide exp
                        cc = q * quad
                        st_ps = stps.tile([P, quad, sq_g], F32, tag="st",
                                          name="st_ps")
                        for k in range(quad):
                            nc.tensor.matmul(
                                st_ps[:, k, :], kt_sb[:, cc + k, :], qg,
                                start=True, stop=True,
                                skip_group_check=True)
                        nc.scalar.activation(pt[:, cc:cc + quad, :],
                                             st_ps[:], Exp, scale=scale)

                    def av(q, pt=pt, ls=ls, ot_ps=ot_ps):
                        # one AV quad (+ its DVE pair-sum every other quad)
                        cc = q * quad
                        if (cc // quad) % 2 == 1:
                            nc.vector.tensor_tensor(
                                ls[:, (cc - quad) // 2:
                                   (cc - quad) // 2 + quad, :],
                                pt[:, cc - quad:cc, :], pt[:, cc:cc + quad, :],
                                mybir.AluOpType.add)
                        for k in range(quad):
                            c = cc + k
                            nc.tensor.matmul(
                                ot_ps[:], v_sb[:, cmap[c], :], pt[:, c, :],
                                start=(c == 0), stop=(c == n_t - 1),
                                skip_group_check=True)

                    def fin(g=g, ls=ls, l_ps=l_ps, ot_ps=ot_ps):
                        w = n_pairs
                        while w > 1:
                            nc.vector.tensor_tensor(
                                ls[:, 0:w // 2, :], ls[:, 0:w // 2, :],
                                ls[:, w // 2:w, :],
                                mybir.AluOpType.add)
                            w //= 2
                        nc.tensor.matmul(
                            l_ps[:], ones_col[:], ls[:, 0, :],
                            start=True, stop=True, skip_group_check=True)
                        # 1/l as a per-partition column, then scale +
                        # transpose out
                        l_sb = p2s.tile([1, sq_g], F32, tag="lsb")
                        nc.vector.tensor_copy(l_sb[:], l_ps[:])
                        ot_sb = p2s.tile([P, sq_g], F32, tag="otsb")
                        nc.vector.tensor_copy(ot_sb[:], ot_ps[:])
                        for j in range(sq_g // P):
                            lc_ps = stps.tile([P, 1], F32, tag="st",
                                              name="lc_ps")
                            nc.tensor.transpose(
                                lc_ps[:], l_sb[0:1, j * P:(j + 1) * P],
                                ident[0:1, 0:1])
                            r_col = p2s.tile([P, 1], F32, tag="rcol",
                                             name="r_col")
                            nc.vector.reciprocal(r_col[:], lc_ps[:])
                            o_tr = stps.tile([P, P], F32, tag="st",
                                             name="o_tr")
                            nc.tensor.transpose(
                                o_tr[:], ot_sb[:, j * P:(j + 1) * P],
                                ident[:])
                            o_sb = p2s.tile([P, head], F32, tag="osb",
                                            name="o_sb")
                            nc.vector.tensor_scalar_mul(
                                o_sb[:], o_tr[:], r_col[:, 0:1])
                            row0 = g * sq_g + j * P
                            nc.sync.dma_start(out[row0:row0 + P, :],
                                              o_sb[:])

                    return sc, av, fin

                assert n_g == 4 and n_quads == 16
                hq = n_quads // 2
                grp = [make_group(g) for g in range(n_g)]
                (sa, aa, fa), (sb, ab, fb) = grp[0], grp[1]
                (scg, ac, fc), (sd, ad, fd) = grp[2], grp[3]
                # pair-1 scores (interleaved halves pipeline PE vs ACT)
                for q in range(hq):
                    sa(q)
                for q in range(hq):
                    sb(q)
                for q in range(hq, n_quads):
                    sa(q)
                for q in range(hq, n_quads):
                    sb(q)
                # g2 first half: V lands somewhere in here
                for q in range(hq):
                    scg(q)
                # remaining scores with pair-1 AV interleaved 1:1
                avq = [(aa, q) for q in range(n_quads)] + \
                    [(ab, q) for q in range(n_quads)]
                for s_fn, q0 in ((sd, 0), (scg, hq), (sd, hq)):
                    for q in range(q0, q0 + hq):
                        s_fn(q)
                        if avq:
                            f, i = avq.pop(0)
                            f(i)
                for f, i in avq:
                    f(i)
                fa()
                fb()
                # pair-2 AV + finishes
                for q in range(n_quads):
                    ac(q)
                for q in range(n_quads):
                    ad(q)
                fc()
                fd()
    nc.compile()
    return nc


_CACHE = {}


def _get_nc():
    if "nc" not in _CACHE:
        nc = bacc.Bacc("TRN2", target_bir_lowering=False, debug=False,
                       num_devices=NCORES)
        _CACHE["nc"] = emit(nc)
    return _CACHE["nc"]


def make_in_maps(x, Wq, bq, Wk, bk, Wv, bv):
    import ml_dtypes
    bf = ml_dtypes.bfloat16
    x = np.ascontiguousarray(np.asarray(x, dtype=np.float32).astype(bf))
    Wq = np.ascontiguousarray(np.asarray(Wq, dtype=np.float32).astype(bf))
    Wk = np.ascontiguousarray(np.asarray(Wk, dtype=np.float32).astype(bf))
    Wv = np.ascontiguousarray(np.asarray(Wv, dtype=np.float32).astype(bf))
    bq = np.ascontiguousarray(np.asarray(bq, dtype=np.float32))
    bk = np.ascontiguousarray(np.asarray(bk, dtype=np.float32))
    bv = np.ascontiguousarray(np.asarray(bv, dtype=np.float32))
    s_loc = SEQ // NCORES
    return [
        {
            "x": np.ascontiguousarray(x[c * s_loc:(c + 1) * s_loc]),
            "wq": Wq, "wk": Wk, "wv": Wv,
            "bq": bq, "bk": bk, "bv": bv,
        }
        for c in range(NCORES)
    ]


def kernel(x, Wq, bq, Wk, bk, Wv, bv):
    in_maps = make_in_maps(x, Wq, bq, Wk, bk, Wv, bv)
    res = run_bass_kernel_spmd(_get_nc(), in_maps,
                               core_ids=list(range(NCORES)))
    return np.concatenate(
        [res.results[c]["out"] for c in range(NCORES)], axis=0)
